# revision 4
# baseline (speedup 1.0000x reference)
"""Trainium2 Bass kernel for nn_LoopModel2: out = x + sum(range(y)).

The loop `for i in range(y): x = x + i` collapses to a single elementwise
add of the constant y*(y-1)/2 (2016.0 for y=64). The kernel is a pure
HBM-streaming problem: DMA tiles of x into SBUF, add the constant, DMA
back out. x (8192, 8192) f32 is sharded row-wise across 8 NeuronCores;
no communication is needed.
"""

import os

import numpy as np

import concourse.bacc as bacc
import concourse.mybir as mybir
from concourse.tile import TileContext
from concourse.bass_utils import run_bass_kernel_spmd

N_CORES = 8
ROWS, COLS = 8192, 8192
SHARD_ROWS = ROWS // N_CORES  # 1024 rows per core

# Tiling of one core's 32 MiB shard: NT tiles of [P, F] f32.
P = 128
F = 8192
NT = (SHARD_ROWS * COLS) // (P * F)  # 8
BUFS = 5

# Filled in by the last traced run (test.py reads this).
LAST_EXEC_NS = None

_cache = {}


def _build(const: float):
    # Bacc (not raw Bass): its finalize() runs generate_event_semaphores,
    # which splits multi-semaphore waits off DMA/compute instructions —
    # walrus codegen rejects >1 inline sync wait per instruction.
    nc = bacc.Bacc()
    x_in = nc.dram_tensor("x", [NT, P, F], mybir.dt.float32, kind="ExternalInput")
    out = nc.dram_tensor("out", [NT, P, F], mybir.dt.float32, kind="ExternalOutput")

    with TileContext(nc) as tc:
        with tc.tile_pool(name="io", bufs=BUFS) as pool:
            for i in range(NT):
                t = pool.tile([P, F], mybir.dt.float32)
                nc.sync.dma_start(out=t[:], in_=x_in[i])
                nc.vector.tensor_scalar_add(t[:], t[:], const)
                nc.sync.dma_start(out=out[i], in_=t[:])
    nc.finalize()
    return nc


def kernel(x, y) -> np.ndarray:
    global LAST_EXEC_NS
    y = int(y)
    const = float(y * (y - 1) // 2)

    if const not in _cache:
        _cache[const] = _build(const)
    nc = _cache[const]

    x_np = np.asarray(x, dtype=np.float32)
    in_maps = [
        {"x": x_np[c * SHARD_ROWS:(c + 1) * SHARD_ROWS].reshape(NT, P, F)}
        for c in range(N_CORES)
    ]
    trace = bool(os.environ.get("KERNEL_TRACE"))
    res = run_bass_kernel_spmd(nc, in_maps, list(range(N_CORES)), trace=trace)
    LAST_EXEC_NS = res.exec_time_ns

    out = np.empty((ROWS, COLS), dtype=np.float32)
    for c in range(N_CORES):
        out[c * SHARD_ROWS:(c + 1) * SHARD_ROWS] = (
            res.results[c]["out"].reshape(SHARD_ROWS, COLS)
        )
    return out


# revision 7
# speedup vs baseline: 1.0031x; 1.0031x over previous
"""Trainium2 Bass kernel for nn_LoopModel2: out = x + sum(range(y)).

The loop `for i in range(y): x = x + i` collapses to a single elementwise
add of the constant y*(y-1)/2 (2016.0 for y=64). The kernel is a pure
HBM-streaming problem: DMA tiles of x into SBUF, add the constant, DMA
back out. x (8192, 8192) f32 is sharded row-wise across 8 NeuronCores;
no communication is needed.
"""

import os

import numpy as np

import concourse.bacc as bacc
import concourse.mybir as mybir
from concourse.tile import TileContext
from concourse.bass_utils import run_bass_kernel_spmd

N_CORES = 8
ROWS, COLS = 8192, 8192
SHARD_ROWS = ROWS // N_CORES  # 1024 rows per core

# Tiling of one core's 32 MiB shard: NT tiles of [P, F] f32.
P = 128
F = 8192
NT = (SHARD_ROWS * COLS) // (P * F)  # 8
BUFS = 5

# Filled in by the last traced run (test.py reads this).
LAST_EXEC_NS = None
LAST_RESULTS = None

_cache = {}


def _build(const: float):
    # Bacc (not raw Bass): its finalize() runs generate_event_semaphores,
    # which splits multi-semaphore waits off DMA/compute instructions —
    # walrus codegen rejects >1 inline sync wait per instruction.
    nc = bacc.Bacc()
    x_in = nc.dram_tensor("x", [NT, P, F], mybir.dt.float32, kind="ExternalInput")
    out = nc.dram_tensor("out", [NT, P, F], mybir.dt.float32, kind="ExternalOutput")

    with TileContext(nc) as tc:
        with tc.tile_pool(name="io", bufs=BUFS) as pool:
            for i in range(NT):
                t = pool.tile([P, F], mybir.dt.float32)
                nc.sync.dma_start(out=t[:], in_=x_in[i])
                nc.vector.tensor_scalar_add(t[:], t[:], const)
                nc.sync.dma_start(out=out[i], in_=t[:])
    nc.finalize()
    return nc


def kernel(x, y) -> np.ndarray:
    global LAST_EXEC_NS, LAST_RESULTS
    y = int(y)
    const = float(y * (y - 1) // 2)

    if const not in _cache:
        _cache[const] = _build(const)
    nc = _cache[const]

    x_np = np.asarray(x, dtype=np.float32)
    in_maps = [
        {"x": x_np[c * SHARD_ROWS:(c + 1) * SHARD_ROWS].reshape(NT, P, F)}
        for c in range(N_CORES)
    ]
    trace = bool(os.environ.get("KERNEL_TRACE"))
    res = run_bass_kernel_spmd(nc, in_maps, list(range(N_CORES)), trace=trace)
    LAST_EXEC_NS = res.exec_time_ns
    LAST_RESULTS = res

    out = np.empty((ROWS, COLS), dtype=np.float32)
    for c in range(N_CORES):
        out[c * SHARD_ROWS:(c + 1) * SHARD_ROWS] = (
            res.results[c]["out"].reshape(SHARD_ROWS, COLS)
        )
    return out


# revision 8
# speedup vs baseline: 1.1425x; 1.1390x over previous
"""Trainium2 Bass kernel for nn_LoopModel2: out = x + sum(range(y)).

The loop `for i in range(y): x = x + i` collapses to a single elementwise
add of the constant y*(y-1)/2 (2016.0 for y=64). The kernel is a pure
HBM-streaming problem: DMA tiles of x into SBUF, add the constant, DMA
back out. x (8192, 8192) f32 is sharded row-wise across 8 NeuronCores;
no communication is needed.
"""

import os

import numpy as np

import concourse.bacc as bacc
import concourse.mybir as mybir
from concourse.tile import TileContext
from concourse.bass_utils import run_bass_kernel_spmd

N_CORES = 8
ROWS, COLS = 8192, 8192
SHARD_ROWS = ROWS // N_CORES  # 1024 rows per core

# Tiling of one core's 32 MiB shard: NT tiles of [P, F] f32.
P = 128
F = int(os.environ.get("KF", 8192))
NT = (SHARD_ROWS * COLS) // (P * F)
BUFS = int(os.environ.get("KBUFS", 5))
# Loads ride the SP HWDGE ring (nc.sync); stores the ACT ring
# (nc.scalar) so both queue rows feed the 16 SDMA engines.
STORE_ENG = os.environ.get("KSTORE", "scalar")

# Filled in by the last traced run (test.py reads this).
LAST_EXEC_NS = None
LAST_RESULTS = None

_cache = {}


def _build(const: float):
    # Bacc (not raw Bass): its finalize() runs generate_event_semaphores,
    # which splits multi-semaphore waits off DMA/compute instructions —
    # walrus codegen rejects >1 inline sync wait per instruction.
    nc = bacc.Bacc()
    x_in = nc.dram_tensor("x", [NT, P, F], mybir.dt.float32, kind="ExternalInput")
    out = nc.dram_tensor("out", [NT, P, F], mybir.dt.float32, kind="ExternalOutput")

    with TileContext(nc) as tc:
        with tc.tile_pool(name="io", bufs=BUFS) as pool:
            for i in range(NT):
                t = pool.tile([P, F], mybir.dt.float32)
                nc.sync.dma_start(out=t[:], in_=x_in[i])
                nc.vector.tensor_scalar_add(t[:], t[:], const)
                getattr(nc, STORE_ENG).dma_start(out=out[i], in_=t[:])
    nc.finalize()
    return nc


def kernel(x, y) -> np.ndarray:
    global LAST_EXEC_NS, LAST_RESULTS
    y = int(y)
    const = float(y * (y - 1) // 2)

    if const not in _cache:
        _cache[const] = _build(const)
    nc = _cache[const]

    x_np = np.asarray(x, dtype=np.float32)
    in_maps = [
        {"x": x_np[c * SHARD_ROWS:(c + 1) * SHARD_ROWS].reshape(NT, P, F)}
        for c in range(N_CORES)
    ]
    trace = bool(os.environ.get("KERNEL_TRACE"))
    res = run_bass_kernel_spmd(nc, in_maps, list(range(N_CORES)), trace=trace)
    LAST_EXEC_NS = res.exec_time_ns
    LAST_RESULTS = res

    out = np.empty((ROWS, COLS), dtype=np.float32)
    for c in range(N_CORES):
        out[c * SHARD_ROWS:(c + 1) * SHARD_ROWS] = (
            res.results[c]["out"].reshape(SHARD_ROWS, COLS)
        )
    return out
